# revision 2
# baseline (speedup 1.0000x reference)
"""Trainium2 Bass kernel for nn_Conv2d_ONI (1x1 conv with ONI-orthogonalized weight).

Strategy:
  - Data-parallel: shard x [32,64,128,128] over batch across 8 NeuronCores
    (4 images each); z/g/bias replicated; ONI (Newton-Schulz on 64x64)
    recomputed on every core (microscopic vs the conv).
  - Per core, the 1x1 conv is a 64x64 channel matmul over 4*128*128 positions.
    Image pairs are stacked on SBUF partitions (partitions 0-63 = channels of
    the even image, 64-127 = odd image) so every DMA uses all 128 partitions
    (full port bandwidth) and the two 64x64 matmuls run concurrently in
    opposite quadrants of the PE array via tile_position packing.
  - The kernel is HBM-bound (~34 MB I/O per core vs ~0.5 GFLOP), so the loop
    streams 2 MiB granules with deep double-buffering on loads (sync/SP ring)
    and stores (scalar/ACT ring).
"""

import sys

for _p in ("/opt/trn_rl_repo",):
    if _p not in sys.path:
        sys.path.insert(0, _p)

import numpy as np

import concourse.bass as bass  # noqa: F401  (needed for engine registration)
import concourse.mybir as mybir
import concourse.tile as tile
from concourse import bacc
from concourse.bass_utils import run_bass_kernel_spmd

F32 = mybir.dt.float32
AL = mybir.AluOpType
SQRT2 = float(np.sqrt(2.0))

N_CORES = 8
N_FULL = 32           # full batch
NB = N_FULL // N_CORES  # images per core (4)
C = 64                # in = out channels
H = W = 128
HW = H * W            # 16384 positions per image
GR = 4096             # granule free size (2 MiB per [128, GR] f32 tile)
ONI_ITR = 5


def _build():
    nc = bacc.Bacc("TRN2", target_bir_lowering=False, debug=False)

    x_h = nc.dram_tensor("x", [NB, C, H, W], F32, kind="ExternalInput")
    z_h = nc.dram_tensor("z", [C, C], F32, kind="ExternalInput")
    g_h = nc.dram_tensor("g", [C, 1], F32, kind="ExternalInput")
    b_h = nc.dram_tensor("bias", [C], F32, kind="ExternalInput")
    eye_h = nc.dram_tensor("eye", [C, C], F32, kind="ExternalInput")
    eye15_h = nc.dram_tensor("eye15", [C, C], F32, kind="ExternalInput")
    onesc_h = nc.dram_tensor("onesc", [C, 1], F32, kind="ExternalInput")
    onesr_h = nc.dram_tensor("onesr", [1, C], F32, kind="ExternalInput")
    y_h = nc.dram_tensor("out", [NB, C, H, W], F32, kind="ExternalOutput")

    # [NB, C, H, W] -> [NB/2, 128, HW]: image pairs stacked on partitions.
    xv = x_h[:].rearrange("(n2 two) c h w -> n2 (two c) (h w)", two=2)
    yv = y_h[:].rearrange("(n2 two) c h w -> n2 (two c) (h w)", two=2)

    with tile.TileContext(nc) as tc:
        with tc.tile_pool(name="consts", bufs=1) as sb, \
             tc.tile_pool(name="nsit", bufs=2) as it, \
             tc.tile_pool(name="xp", bufs=4) as xp, \
             tc.tile_pool(name="op", bufs=4) as op, \
             tc.tile_pool(name="onips", bufs=3, space="PSUM") as psp, \
             tc.tile_pool(name="wps", bufs=1, space="PSUM") as wpsp, \
             tc.tile_pool(name="convps", bufs=4, space="PSUM") as cpsp:

            # ---- load params + constants ----
            z_sb = sb.tile([C, C], F32)
            nc.sync.dma_start(out=z_sb, in_=z_h[:])
            g_sb = sb.tile([C, 1], F32)
            nc.sync.dma_start(out=g_sb, in_=g_h[:])
            eye_sb = sb.tile([C, C], F32)
            nc.sync.dma_start(out=eye_sb, in_=eye_h[:])
            eye15_sb = sb.tile([C, C], F32)
            nc.sync.dma_start(out=eye15_sb, in_=eye15_h[:])
            onesc_sb = sb.tile([C, 1], F32)
            nc.sync.dma_start(out=onesc_sb, in_=onesc_h[:])
            onesr_sb = sb.tile([1, C], F32)
            nc.sync.dma_start(out=onesr_sb, in_=onesr_h[:])
            bias_sb = sb.tile([2 * C, 1], F32)
            bcol = b_h[:].rearrange("(c u) -> c u", u=1)
            nc.sync.dma_start(out=bias_sb[0:C, :], in_=bcol)
            nc.sync.dma_start(out=bias_sb[C : 2 * C, :], in_=bcol)

            # ---- ONI: weight = (NewtonSchulz(center(z))) * g * sqrt(2) ----
            # zc = z - rowmean(z)
            rowsum = sb.tile([C, 1], F32)
            nc.vector.reduce_sum(rowsum, z_sb, axis=mybir.AxisListType.X)
            rowmean = sb.tile([C, 1], F32)
            nc.scalar.mul(rowmean, rowsum, 1.0 / C)
            zc_sb = sb.tile([C, C], F32)
            nc.vector.tensor_scalar_sub(zc_sb, z_sb, rowmean)

            # zcT (PE transpose)
            zcT_ps = psp.tile([C, C], F32, tag="ps")
            nc.tensor.transpose(zcT_ps, zc_sb, eye_sb)
            zcT_sb = sb.tile([C, C], F32)
            nc.vector.tensor_copy(zcT_sb, zcT_ps)

            # s1 = zc @ zc.T
            s1_ps = psp.tile([C, C], F32, tag="ps")
            nc.tensor.matmul(s1_ps, zcT_sb, zcT_sb, start=True, stop=True)
            s1_sb = sb.tile([C, C], F32)
            nc.vector.tensor_copy(s1_sb, s1_ps)

            # fro2 = sum(s1^2) (free-dim reduce, then cross-partition via matmul)
            sq_sb = sb.tile([C, C], F32)
            colsq = sb.tile([C, 1], F32)
            nc.vector.tensor_mul(sq_sb, s1_sb, s1_sb)
            nc.vector.reduce_sum(colsq, sq_sb, axis=mybir.AxisListType.X)
            fro2_ps = psp.tile([1, 1], F32, tag="ps")
            nc.tensor.matmul(fro2_ps, colsq, onesc_sb, start=True, stop=True)
            fro2_sb = sb.tile([1, 1], F32)
            nc.vector.tensor_copy(fro2_sb, fro2_ps)

            # invn = 1/||s1||_F = 1/sqrt(fro2); rs = ||s1||_F^(-1/2) = sqrt(invn)
            t_sb = sb.tile([1, 1], F32)
            nc.scalar.sqrt(t_sb, fro2_sb)
            invn_sb = sb.tile([1, 1], F32)
            nc.vector.reciprocal(invn_sb, t_sb)
            rs_sb = sb.tile([1, 1], F32)
            nc.scalar.sqrt(rs_sb, invn_sb)
            scal2 = sb.tile([1, 2], F32)
            nc.vector.tensor_copy(scal2[:, 0:1], invn_sb)
            nc.vector.tensor_copy(scal2[:, 1:2], rs_sb)
            bc_ps = psp.tile([C, 2], F32, tag="ps")
            nc.tensor.matmul(bc_ps, onesr_sb, scal2, start=True, stop=True)
            bc_sb = sb.tile([C, 2], F32)
            nc.vector.tensor_copy(bc_sb, bc_ps)

            # s = s1 * invn ; v = zc * rs ; b = 1.5 I - 0.5 s
            s_sb = sb.tile([C, C], F32)
            nc.vector.tensor_scalar_mul(s_sb, s1_sb, bc_sb[:, 0:1])
            v_sb = sb.tile([C, C], F32)
            nc.vector.tensor_scalar_mul(v_sb, zc_sb, bc_sb[:, 1:2])
            b_sb = sb.tile([C, C], F32)
            nc.vector.scalar_tensor_tensor(
                out=b_sb, in0=s_sb, scalar=-0.5, in1=eye15_sb,
                op0=AL.mult, op1=AL.add,
            )

            # b <- 1.5 b - 0.5 (b@b)(b@s)   (b, s symmetric; b = poly(s))
            for _ in range(1, ONI_ITR):
                p_ps = psp.tile([C, C], F32, tag="ps")
                nc.tensor.matmul(p_ps, b_sb, b_sb, start=True, stop=True)
                p_sb = it.tile([C, C], F32, tag="p")
                nc.vector.tensor_copy(p_sb, p_ps)
                q_ps = psp.tile([C, C], F32, tag="ps")
                nc.tensor.matmul(q_ps, b_sb, s_sb, start=True, stop=True)
                q_sb = it.tile([C, C], F32, tag="q")
                nc.vector.tensor_copy(q_sb, q_ps)
                r_ps = psp.tile([C, C], F32, tag="ps")
                nc.tensor.matmul(r_ps, p_sb, q_sb, start=True, stop=True)
                rh_sb = it.tile([C, C], F32, tag="rh")
                nc.scalar.mul(rh_sb, r_ps, 0.5)
                b_new = it.tile([C, C], F32, tag="b")
                nc.vector.scalar_tensor_tensor(
                    out=b_new, in0=b_sb, scalar=1.5, in1=rh_sb,
                    op0=AL.mult, op1=AL.subtract,
                )
                b_sb = b_new

            # fold g*sqrt2 into b's columns: gbc[p, o] = g[o]*sqrt2
            gT_ps = psp.tile([1, C], F32, tag="ps")
            nc.tensor.transpose(gT_ps, g_sb, eye_sb)
            gT_sb = sb.tile([1, C], F32)
            nc.scalar.mul(gT_sb, gT_ps, SQRT2)
            gb_ps = psp.tile([C, C], F32, tag="ps")
            nc.tensor.matmul(gb_ps, onesr_sb, gT_sb, start=True, stop=True)
            gbc_sb = sb.tile([C, C], F32)
            nc.vector.tensor_copy(gbc_sb, gb_ps)
            bg_sb = sb.tile([C, C], F32)
            nc.vector.tensor_mul(bg_sb, b_sb, gbc_sb)

            # weight^T = v^T @ (b*g*sqrt2), replicated on both partition halves
            w_ps = wpsp.tile([2 * C, C], F32)
            nc.tensor.matmul(w_ps[0:C, :], v_sb, bg_sb,
                             start=True, stop=True, tile_position=(0, 0))
            nc.tensor.matmul(w_ps[C : 2 * C, :], v_sb, bg_sb,
                             start=True, stop=True, tile_position=(0, C))
            wT_sb = sb.tile([2 * C, C], F32)
            nc.vector.tensor_copy(wT_sb, w_ps)

            # ---- conv: stream x, y = W @ x + bias ----
            for n2 in range(NB // 2):
                for gi in range(HW // GR):
                    lo = gi * GR
                    xt = xp.tile([2 * C, GR], F32)
                    nc.sync.dma_start(out=xt, in_=xv[n2, :, lo : lo + GR])
                    ot = op.tile([2 * C, GR], F32)
                    for j in range(GR // 512):
                        sl = slice(j * 512, (j + 1) * 512)
                        ps = cpsp.tile([2 * C, 512], F32)
                        nc.tensor.matmul(ps[0:C, :], wT_sb[0:C, :], xt[0:C, sl],
                                         start=True, stop=True, tile_position=(0, 0))
                        nc.tensor.matmul(ps[C : 2 * C, :], wT_sb[C : 2 * C, :],
                                         xt[C : 2 * C, sl],
                                         start=True, stop=True, tile_position=(C, C))
                        nc.vector.tensor_scalar_add(ot[:, sl], ps, bias_sb)
                    nc.scalar.dma_start(out=yv[n2, :, lo : lo + GR], in_=ot)

    nc.compile()
    return nc


_NC_CACHE = None


def _get_nc():
    global _NC_CACHE
    if _NC_CACHE is None:
        _NC_CACHE = _build()
    return _NC_CACHE


def _run(inputs, trace=False, **spmd_kwargs):
    nc = _get_nc()
    x = np.ascontiguousarray(np.asarray(inputs["x"], dtype=np.float32))
    z = np.ascontiguousarray(np.asarray(inputs["z"], dtype=np.float32))
    g = np.ascontiguousarray(np.asarray(inputs["g"], dtype=np.float32))
    bias = np.ascontiguousarray(np.asarray(inputs["bias"], dtype=np.float32))

    eye = np.eye(C, dtype=np.float32)
    eye15 = (1.5 * np.eye(C)).astype(np.float32)
    onesc = np.ones((C, 1), np.float32)
    onesr = np.ones((1, C), np.float32)

    in_maps = []
    for i in range(N_CORES):
        in_maps.append({
            "x": x[i * NB : (i + 1) * NB],
            "z": z, "g": g, "bias": bias,
            "eye": eye, "eye15": eye15, "onesc": onesc, "onesr": onesr,
        })
    res = run_bass_kernel_spmd(nc, in_maps, core_ids=list(range(N_CORES)),
                               trace=trace, **spmd_kwargs)
    out = np.concatenate([res.results[i]["out"] for i in range(N_CORES)], axis=0)
    return out, res


def kernel(**inputs) -> np.ndarray:
    out, _ = _run(inputs)
    return out


# revision 4
# speedup vs baseline: 1.0601x; 1.0601x over previous
"""Trainium2 Bass kernel for nn_Conv2d_ONI (1x1 conv with ONI-orthogonalized weight).

Strategy:
  - Data-parallel: shard x [32,64,128,128] over batch across 8 NeuronCores
    (4 images each); z/g/bias replicated; ONI (Newton-Schulz on 64x64)
    recomputed on every core (microscopic vs the conv).
  - Per core, the 1x1 conv is a 64x64 channel matmul over 4*128*128 positions.
    Image pairs are stacked on SBUF partitions (partitions 0-63 = channels of
    the even image, 64-127 = odd image) so every DMA uses all 128 partitions
    (full port bandwidth) and the two 64x64 matmuls run concurrently in
    opposite quadrants of the PE array via tile_position packing.
  - The kernel is HBM-bound (~34 MB I/O per core vs ~0.5 GFLOP), so the loop
    streams 2 MiB granules with deep double-buffering on loads (sync/SP ring)
    and stores (scalar/ACT ring).
"""

import sys

for _p in ("/opt/trn_rl_repo",):
    if _p not in sys.path:
        sys.path.insert(0, _p)

import numpy as np

import concourse.bass as bass  # noqa: F401  (needed for engine registration)
import concourse.mybir as mybir
import concourse.tile as tile
from concourse import bacc
from concourse.bass_utils import run_bass_kernel_spmd

F32 = mybir.dt.float32
AL = mybir.AluOpType
SQRT2 = float(np.sqrt(2.0))

N_CORES = 8
N_FULL = 32           # full batch
NB = N_FULL // N_CORES  # images per core (4)
C = 64                # in = out channels
H = W = 128
HW = H * W            # 16384 positions per image
GR = 4096             # granule free size (2 MiB per [128, GR] f32 tile)
ONI_ITR = 5


def _build():
    nc = bacc.Bacc("TRN2", target_bir_lowering=False, debug=False)

    x_h = nc.dram_tensor("x", [NB, C, H, W], F32, kind="ExternalInput")
    z_h = nc.dram_tensor("z", [C, C], F32, kind="ExternalInput")
    g_h = nc.dram_tensor("g", [C, 1], F32, kind="ExternalInput")
    b_h = nc.dram_tensor("bias", [C], F32, kind="ExternalInput")
    eye_h = nc.dram_tensor("eye", [C, C], F32, kind="ExternalInput")
    eye15_h = nc.dram_tensor("eye15", [C, C], F32, kind="ExternalInput")
    onesc_h = nc.dram_tensor("onesc", [C, 1], F32, kind="ExternalInput")
    onesr_h = nc.dram_tensor("onesr", [1, C], F32, kind="ExternalInput")
    y_h = nc.dram_tensor("out", [NB, C, H, W], F32, kind="ExternalOutput")

    # [NB, C, H, W] -> [NB/2, 128, HW]: image pairs stacked on partitions.
    xv = x_h[:].rearrange("(n2 two) c h w -> n2 (two c) (h w)", two=2)
    yv = y_h[:].rearrange("(n2 two) c h w -> n2 (two c) (h w)", two=2)

    with tile.TileContext(nc) as tc:
        with tc.tile_pool(name="consts", bufs=1) as sb, \
             tc.tile_pool(name="nsit", bufs=2) as it, \
             tc.tile_pool(name="xp", bufs=6) as xp, \
             tc.tile_pool(name="op", bufs=4) as op, \
             tc.tile_pool(name="onips", bufs=3, space="PSUM") as psp, \
             tc.tile_pool(name="wps", bufs=1, space="PSUM") as wpsp, \
             tc.tile_pool(name="convps", bufs=4, space="PSUM") as cpsp:

            # ---- load params + constants (scalar/ACT ring: keeps the sync
            # ring free so x-granule loads start immediately) ----
            z_sb = sb.tile([C, C], F32)
            nc.scalar.dma_start(out=z_sb, in_=z_h[:])
            eye_sb = sb.tile([C, C], F32)
            nc.scalar.dma_start(out=eye_sb, in_=eye_h[:])
            g_sb = sb.tile([C, 1], F32)
            nc.scalar.dma_start(out=g_sb, in_=g_h[:])
            eye15_sb = sb.tile([C, C], F32)
            nc.scalar.dma_start(out=eye15_sb, in_=eye15_h[:])
            onesc_sb = sb.tile([C, 1], F32)
            nc.scalar.dma_start(out=onesc_sb, in_=onesc_h[:])
            onesr_sb = sb.tile([1, C], F32)
            nc.scalar.dma_start(out=onesr_sb, in_=onesr_h[:])
            bias_sb = sb.tile([2 * C, 1], F32)
            bcol = b_h[:].rearrange("(c u) -> c u", u=1)
            nc.scalar.dma_start(out=bias_sb[0:C, :], in_=bcol)
            nc.scalar.dma_start(out=bias_sb[C : 2 * C, :], in_=bcol)

            # ---- ONI: weight = (NewtonSchulz(center(z))) * g * sqrt(2) ----
            # All intermediate scalings by powers of two cancel exactly:
            # Newton-Schulz input s = s1/||s1|| and v = zc*||s1||^-1/2 are
            # invariant to zc -> 64*zc, so center via zc' = 64*z - rowsum
            # (one DVE op, no 1/64 mean step).
            rowsum = sb.tile([C, 1], F32)
            nc.vector.reduce_sum(rowsum, z_sb, axis=mybir.AxisListType.X)
            zc_sb = sb.tile([C, C], F32)
            nc.vector.tensor_scalar(zc_sb, z_sb, float(C), rowsum,
                                    op0=AL.mult, op1=AL.subtract)

            # zcT (PE transpose)
            zcT_ps = psp.tile([C, C], F32, tag="ps")
            nc.tensor.transpose(zcT_ps, zc_sb, eye_sb)
            zcT_sb = sb.tile([C, C], F32)
            nc.vector.tensor_copy(zcT_sb, zcT_ps)

            # s1 = zc @ zc.T
            s1_ps = psp.tile([C, C], F32, tag="ps")
            nc.tensor.matmul(s1_ps, zcT_sb, zcT_sb, start=True, stop=True)
            s1_sb = sb.tile([C, C], F32)
            nc.vector.tensor_copy(s1_sb, s1_ps)

            # fro2 = sum(s1^2): ACT square+row-accumulate straight from PSUM
            # (parallel to the DVE copy above), then cross-partition matmul.
            sq_sb = sb.tile([C, C], F32)
            colsq = sb.tile([C, 1], F32)
            nc.scalar.activation(out=sq_sb, in_=s1_ps,
                                 func=mybir.ActivationFunctionType.Square,
                                 accum_out=colsq)
            fro2_ps = psp.tile([1, 1], F32, tag="ps")
            nc.tensor.matmul(fro2_ps, colsq, onesc_sb, start=True, stop=True)

            # invn = 1/||s1||_F = 1/sqrt(fro2)  (sqrt reads PSUM directly)
            t_sb = sb.tile([1, 1], F32)
            nc.scalar.sqrt(t_sb, fro2_ps)
            invn_sb = sb.tile([1, 1], F32)
            nc.vector.reciprocal(invn_sb, t_sb)

            # broadcast invn across partitions (K=1 matmul), use from PSUM
            bc_ps = psp.tile([C, 1], F32, tag="ps")
            nc.tensor.matmul(bc_ps, onesr_sb, invn_sb, start=True, stop=True)

            # s = s1 * invn ; b = 1.5 I - 0.5 s
            s_sb = sb.tile([C, C], F32)
            nc.vector.tensor_scalar_mul(s_sb, s1_sb, bc_ps[:, 0:1])
            b_sb = sb.tile([C, C], F32)
            nc.vector.scalar_tensor_tensor(
                out=b_sb, in0=s_sb, scalar=-0.5, in1=eye15_sb,
                op0=AL.mult, op1=AL.add,
            )

            # b <- 1.5 b - 0.5 (b@b)(b@s)   (b, s symmetric; b = poly(s))
            for _ in range(1, ONI_ITR):
                p_ps = psp.tile([C, C], F32, tag="ps")
                nc.tensor.matmul(p_ps, b_sb, b_sb, start=True, stop=True)
                q_ps = psp.tile([C, C], F32, tag="ps")
                nc.tensor.matmul(q_ps, b_sb, s_sb, start=True, stop=True)
                ph_sb = it.tile([C, C], F32, tag="ph")
                nc.scalar.mul(ph_sb, p_ps, -0.5)       # ACT: -(1/2) p, PSUM in
                q_sb = it.tile([C, C], F32, tag="q")
                nc.vector.tensor_copy(q_sb, q_ps)      # DVE, parallel with ACT
                r_ps = psp.tile([C, C], F32, tag="ps")
                nc.tensor.matmul(r_ps, ph_sb, q_sb, start=True, stop=True)
                b_new = it.tile([C, C], F32, tag="b")
                nc.vector.scalar_tensor_tensor(        # 1.5 b + r  (r from PSUM)
                    out=b_new, in0=b_sb, scalar=1.5, in1=r_ps,
                    op0=AL.mult, op1=AL.add,
                )
                b_sb = b_new

            # g scaling column broadcast, with rs*sqrt2 folded into the scale:
            # rs = sqrt(invn), so rs*sqrt2 = sqrt(2*invn). The 64x zc scaling
            # cancels through invn exactly. (off the critical NS path)
            rs2_sb = sb.tile([1, 1], F32)
            nc.scalar.activation(out=rs2_sb, in_=invn_sb,
                                 func=mybir.ActivationFunctionType.Sqrt,
                                 scale=2.0)
            gT_ps = psp.tile([1, C], F32, tag="ps")
            nc.tensor.transpose(gT_ps, g_sb, eye_sb)
            gT_sb = sb.tile([1, C], F32)
            nc.scalar.activation(out=gT_sb, in_=gT_ps,
                                 func=mybir.ActivationFunctionType.Copy,
                                 scale=rs2_sb[0:1, 0:1])
            gb_ps = psp.tile([C, C], F32, tag="ps")
            nc.tensor.matmul(gb_ps, onesr_sb, gT_sb, start=True, stop=True)
            gbc_sb = sb.tile([C, C], F32)
            nc.vector.tensor_copy(gbc_sb, gb_ps)
            bg_sb = sb.tile([C, C], F32)
            nc.vector.tensor_mul(bg_sb, b_sb, gbc_sb)
            v_sb = zc_sb  # rs folded into gbc; zc' self-normalizes (see above)

            # weight^T = v^T @ (b*g*sqrt2), replicated on both partition halves
            w_ps = wpsp.tile([2 * C, C], F32)
            nc.tensor.matmul(w_ps[0:C, :], v_sb, bg_sb,
                             start=True, stop=True, tile_position=(0, 0))
            nc.tensor.matmul(w_ps[C : 2 * C, :], v_sb, bg_sb,
                             start=True, stop=True, tile_position=(0, C))
            wT_sb = sb.tile([2 * C, C], F32)
            nc.vector.tensor_copy(wT_sb, w_ps)

            # ---- conv: stream x, y = W @ x + bias ----
            for n2 in range(NB // 2):
                for gi in range(HW // GR):
                    lo = gi * GR
                    xt = xp.tile([2 * C, GR], F32)
                    nc.sync.dma_start(out=xt, in_=xv[n2, :, lo : lo + GR])
                    ot = op.tile([2 * C, GR], F32)
                    for j in range(GR // 512):
                        sl = slice(j * 512, (j + 1) * 512)
                        ps = cpsp.tile([2 * C, 512], F32)
                        nc.tensor.matmul(ps[0:C, :], wT_sb[0:C, :], xt[0:C, sl],
                                         start=True, stop=True, tile_position=(0, 0))
                        nc.tensor.matmul(ps[C : 2 * C, :], wT_sb[C : 2 * C, :],
                                         xt[C : 2 * C, sl],
                                         start=True, stop=True, tile_position=(C, C))
                        nc.vector.tensor_scalar_add(ot[:, sl], ps, bias_sb)
                    nc.scalar.dma_start(out=yv[n2, :, lo : lo + GR], in_=ot)

    nc.compile()
    return nc


_NC_CACHE = None


def _get_nc():
    global _NC_CACHE
    if _NC_CACHE is None:
        _NC_CACHE = _build()
    return _NC_CACHE


def _run(inputs, trace=False, **spmd_kwargs):
    nc = _get_nc()
    x = np.ascontiguousarray(np.asarray(inputs["x"], dtype=np.float32))
    z = np.ascontiguousarray(np.asarray(inputs["z"], dtype=np.float32))
    g = np.ascontiguousarray(np.asarray(inputs["g"], dtype=np.float32))
    bias = np.ascontiguousarray(np.asarray(inputs["bias"], dtype=np.float32))

    eye = np.eye(C, dtype=np.float32)
    eye15 = (1.5 * np.eye(C)).astype(np.float32)
    onesc = np.ones((C, 1), np.float32)
    onesr = np.ones((1, C), np.float32)

    in_maps = []
    for i in range(N_CORES):
        in_maps.append({
            "x": x[i * NB : (i + 1) * NB],
            "z": z, "g": g, "bias": bias,
            "eye": eye, "eye15": eye15, "onesc": onesc, "onesr": onesr,
        })
    res = run_bass_kernel_spmd(nc, in_maps, core_ids=list(range(N_CORES)),
                               trace=trace, **spmd_kwargs)
    out = np.concatenate([res.results[i]["out"] for i in range(N_CORES)], axis=0)
    return out, res


def kernel(**inputs) -> np.ndarray:
    out, _ = _run(inputs)
    return out


# revision 6
# speedup vs baseline: 1.0887x; 1.0270x over previous
"""Trainium2 Bass kernel for nn_Conv2d_ONI (1x1 conv with ONI-orthogonalized weight).

Strategy:
  - Data-parallel: shard x [32,64,128,128] over batch across 8 NeuronCores
    (4 images each); z/g/bias replicated; ONI (Newton-Schulz on 64x64)
    recomputed on every core (microscopic vs the conv).
  - Per core, the 1x1 conv is a 64x64 channel matmul over 4*128*128 positions.
    Image pairs are stacked on SBUF partitions (partitions 0-63 = channels of
    the even image, 64-127 = odd image) so every DMA uses all 128 partitions
    (full port bandwidth) and the two 64x64 matmuls run concurrently in
    opposite quadrants of the PE array via tile_position packing.
  - The kernel is fabric-bound (~34 MB HBM I/O per core vs ~0.5 GFLOP;
    loads+stores share the ~435 GB/s SBUF-AXI ceiling), so the loop streams
    2 MiB granules with deep double-buffering: loads on the sync/SP HWDGE
    ring, stores on the scalar/ACT ring.
  - All small parameters (z) and host-precomputable constants (identity,
    1.5*identity, g-broadcast, bias, ones) are packed into ONE [128, 322]
    tensor whose single DMA is issued first on the sync ring, so it
    FIFO-completes before the 2 MiB x-granule floods and the ONI serial
    chain starts as early as possible.
"""

import sys

for _p in ("/opt/trn_rl_repo",):
    if _p not in sys.path:
        sys.path.insert(0, _p)

import numpy as np

import concourse.bass as bass  # noqa: F401  (needed for engine registration)
import concourse.mybir as mybir
import concourse.tile as tile
from concourse import bacc
from concourse.bass_utils import run_bass_kernel_spmd

F32 = mybir.dt.float32
AL = mybir.AluOpType
SQRT2 = float(np.sqrt(2.0))

N_CORES = 8
N_FULL = 32           # full batch
NB = N_FULL // N_CORES  # images per core (4)
C = 64                # in = out channels
H = W = 128
HW = H * W            # 16384 positions per image
GR = 4096             # granule free size (2 MiB per [128, GR] f32 tile)
ONI_ITR = 5
PCOLS = 322           # packed parm tensor columns


def _build():
    nc = bacc.Bacc("TRN2", target_bir_lowering=False, debug=False)

    x_h = nc.dram_tensor("x", [NB, C, H, W], F32, kind="ExternalInput")
    parm_h = nc.dram_tensor("parm", [2 * C, PCOLS], F32, kind="ExternalInput")
    y_h = nc.dram_tensor("out", [NB, C, H, W], F32, kind="ExternalOutput")

    # [NB, C, H, W] -> [NB/2, 128, HW]: image pairs stacked on partitions.
    xv = x_h[:].rearrange("(n2 two) c h w -> n2 (two c) (h w)", two=2)
    yv = y_h[:].rearrange("(n2 two) c h w -> n2 (two c) (h w)", two=2)

    with tile.TileContext(nc) as tc:
        with tc.tile_pool(name="consts", bufs=1) as sb, \
             tc.tile_pool(name="nsit", bufs=2) as it, \
             tc.tile_pool(name="xp", bufs=6) as xp, \
             tc.tile_pool(name="op", bufs=4) as op, \
             tc.tile_pool(name="onips", bufs=3, space="PSUM") as psp, \
             tc.tile_pool(name="wps", bufs=1, space="PSUM") as wpsp, \
             tc.tile_pool(name="convps", bufs=4, space="PSUM") as cpsp:

            # ---- one packed param/const DMA, first on the sync ring ----
            parm_sb = sb.tile([2 * C, PCOLS], F32)
            nc.sync.dma_start(out=parm_sb, in_=parm_h[:])
            z_sb = parm_sb[0:C, 0:C]
            eye_sb = parm_sb[0:C, C : 2 * C]
            eye15_sb = parm_sb[0:C, 2 * C : 3 * C]
            gbc_sb = parm_sb[0:C, 3 * C : 4 * C]       # rows = g^T * sqrt2
            bias_sb = parm_sb[:, 4 * C : 4 * C + 1]    # [128,1]
            onesc_sb = parm_sb[0:C, 4 * C + 1 : 4 * C + 2]
            onesr_sb = parm_sb[0:1, 4 * C + 2 : 5 * C + 2]

            # ---- ONI: weight = (NewtonSchulz(center(z))) * g * sqrt(2) ----
            # Newton-Schulz input s = s1/||s1|| and v = zc*||s1||^-1/2 are
            # invariant under zc -> 64*zc (powers of two cancel exactly), so
            # center via zc' = 64*z - rowsum: one DVE op, no 1/64 mean step.
            rowsum = sb.tile([C, 1], F32)
            nc.vector.reduce_sum(rowsum, z_sb, axis=mybir.AxisListType.X)
            zc_sb = sb.tile([C, C], F32)
            nc.vector.tensor_scalar(zc_sb, z_sb, float(C), rowsum,
                                    op0=AL.mult, op1=AL.subtract)

            # zcT (PE transpose)
            zcT_ps = psp.tile([C, C], F32, tag="ps")
            nc.tensor.transpose(zcT_ps, zc_sb, eye_sb)
            zcT_sb = sb.tile([C, C], F32)
            nc.vector.tensor_copy(zcT_sb, zcT_ps)

            # s1 = zc @ zc.T
            s1_ps = psp.tile([C, C], F32, tag="ps")
            nc.tensor.matmul(s1_ps, zcT_sb, zcT_sb, start=True, stop=True)
            s1_sb = sb.tile([C, C], F32)
            nc.vector.tensor_copy(s1_sb, s1_ps)

            # fro2 = sum(s1^2): ACT square+row-accumulate straight from PSUM
            # (parallel to the DVE copy above), then cross-partition matmul.
            sq_sb = sb.tile([C, C], F32)
            colsq = sb.tile([C, 1], F32)
            nc.scalar.activation(out=sq_sb, in_=s1_ps,
                                 func=mybir.ActivationFunctionType.Square,
                                 accum_out=colsq)
            fro2_ps = psp.tile([1, 1], F32, tag="ps")
            nc.tensor.matmul(fro2_ps, colsq, onesc_sb, start=True, stop=True)

            # invn = 1/||s1||_F = sqrt(1/fro2); rs*sqrt2 = sqrt(2*invn).
            # (DVE reciprocal reads PSUM; both sqrt on ACT back-to-back.)
            rin_sb = sb.tile([1, 1], F32)
            nc.vector.reciprocal(rin_sb, fro2_ps)
            scal2 = sb.tile([1, 2], F32)
            nc.scalar.activation(out=scal2[:, 0:1], in_=rin_sb,
                                 func=mybir.ActivationFunctionType.Sqrt)
            nc.scalar.activation(out=scal2[:, 1:2], in_=scal2[:, 0:1],
                                 func=mybir.ActivationFunctionType.Sqrt,
                                 scale=2.0)
            # broadcast (invn, rs*sqrt2) across partitions via K=1 matmul
            bc_ps = psp.tile([C, 2], F32, tag="ps")
            nc.tensor.matmul(bc_ps, onesr_sb, scal2, start=True, stop=True)

            # s = s1 * invn ; b = 1.5 I - 0.5 s
            s_sb = sb.tile([C, C], F32)
            nc.vector.tensor_scalar_mul(s_sb, s1_sb, bc_ps[:, 0:1])
            b_sb = sb.tile([C, C], F32)
            nc.vector.scalar_tensor_tensor(
                out=b_sb, in0=s_sb, scalar=-0.5, in1=eye15_sb,
                op0=AL.mult, op1=AL.add,
            )

            # b <- 1.5 b - 0.5 (b@b)(b@s)   (b, s symmetric; b = poly(s))
            for _ in range(1, ONI_ITR):
                p_ps = psp.tile([C, C], F32, tag="ps")
                nc.tensor.matmul(p_ps, b_sb, b_sb, start=True, stop=True)
                q_ps = psp.tile([C, C], F32, tag="ps")
                nc.tensor.matmul(q_ps, b_sb, s_sb, start=True, stop=True)
                ph_sb = it.tile([C, C], F32, tag="ph")
                nc.scalar.mul(ph_sb, p_ps, -0.5)       # ACT: -(1/2) p, PSUM in
                q_sb = it.tile([C, C], F32, tag="q")
                nc.vector.tensor_copy(q_sb, q_ps)      # DVE, parallel with ACT
                r_ps = psp.tile([C, C], F32, tag="ps")
                nc.tensor.matmul(r_ps, ph_sb, q_sb, start=True, stop=True)
                b_new = it.tile([C, C], F32, tag="b")
                nc.vector.scalar_tensor_tensor(        # 1.5 b + r  (r from PSUM)
                    out=b_new, in0=b_sb, scalar=1.5, in1=r_ps,
                    op0=AL.mult, op1=AL.add,
                )
                b_sb = b_new

            # bg = b * (g^T*sqrt2 rows) * (rs*sqrt2 ... rs scalar): one DVE op.
            # The 64x zc scaling cancels through invn/rs exactly.
            bg_sb = sb.tile([C, C], F32)
            nc.vector.scalar_tensor_tensor(
                out=bg_sb, in0=b_sb, scalar=bc_ps[:, 1:2], in1=gbc_sb,
                op0=AL.mult, op1=AL.mult,
            )
            v_sb = zc_sb  # rs folded into bg; zc' self-normalizes (see above)

            # weight^T = v^T @ bg, replicated on both partition halves
            w_ps = wpsp.tile([2 * C, C], F32)
            nc.tensor.matmul(w_ps[0:C, :], v_sb, bg_sb,
                             start=True, stop=True, tile_position=(0, 0))
            nc.tensor.matmul(w_ps[C : 2 * C, :], v_sb, bg_sb,
                             start=True, stop=True, tile_position=(0, C))
            wT_sb = sb.tile([2 * C, C], F32)
            nc.vector.tensor_copy(wT_sb, w_ps)

            # ---- conv: stream x, y = W @ x + bias ----
            for n2 in range(NB // 2):
                for gi in range(HW // GR):
                    lo = gi * GR
                    xt = xp.tile([2 * C, GR], F32)
                    nc.sync.dma_start(out=xt, in_=xv[n2, :, lo : lo + GR])
                    ot = op.tile([2 * C, GR], F32)
                    for j in range(GR // 512):
                        sl = slice(j * 512, (j + 1) * 512)
                        ps = cpsp.tile([2 * C, 512], F32)
                        nc.tensor.matmul(ps[0:C, :], wT_sb[0:C, :], xt[0:C, sl],
                                         start=True, stop=True, tile_position=(0, 0))
                        nc.tensor.matmul(ps[C : 2 * C, :], wT_sb[C : 2 * C, :],
                                         xt[C : 2 * C, sl],
                                         start=True, stop=True, tile_position=(C, C))
                        nc.vector.tensor_scalar_add(ot[:, sl], ps, bias_sb)
                    nc.scalar.dma_start(out=yv[n2, :, lo : lo + GR], in_=ot)

    nc.compile()
    return nc


_NC_CACHE = None


def _get_nc():
    global _NC_CACHE
    if _NC_CACHE is None:
        _NC_CACHE = _build()
    return _NC_CACHE


def _make_parm(z, g, bias):
    parm = np.zeros((2 * C, PCOLS), np.float32)
    parm[0:C, 0:C] = z
    parm[0:C, C : 2 * C] = np.eye(C, dtype=np.float32)
    parm[0:C, 2 * C : 3 * C] = (1.5 * np.eye(C)).astype(np.float32)
    parm[0:C, 3 * C : 4 * C] = np.broadcast_to(g.reshape(C)[None, :], (C, C))
    parm[0:C, 4 * C] = bias
    parm[C : 2 * C, 4 * C] = bias
    parm[0:C, 4 * C + 1] = 1.0
    parm[0:1, 4 * C + 2 : 5 * C + 2] = 1.0
    return parm


def _run(inputs, trace=False, **spmd_kwargs):
    nc = _get_nc()
    x = np.ascontiguousarray(np.asarray(inputs["x"], dtype=np.float32))
    z = np.asarray(inputs["z"], dtype=np.float32)
    g = np.asarray(inputs["g"], dtype=np.float32)
    bias = np.asarray(inputs["bias"], dtype=np.float32)
    parm = _make_parm(z, g, bias)

    in_maps = []
    for i in range(N_CORES):
        in_maps.append({"x": x[i * NB : (i + 1) * NB], "parm": parm})
    res = run_bass_kernel_spmd(nc, in_maps, core_ids=list(range(N_CORES)),
                               trace=trace, **spmd_kwargs)
    out = np.concatenate([res.results[i]["out"] for i in range(N_CORES)], axis=0)
    return out, res


def kernel(**inputs) -> np.ndarray:
    out, _ = _run(inputs)
    return out
